# revision 19
# baseline (speedup 1.0000x reference)
"""Trainium2 Bass kernel for nn_Aggregator (segment_reduce):
res[b,d] = sum_n mask[b,n] * (nodes@Wt.T + bt)[n,d] * sigmoid(nodes@Wg.T + bg)[n,d]

Sharding: nodes and owner_masks split along N across 8 NeuronCores; params
replicated; per-core partial [B,D] summed on host.

Host-side prep (part of sharding): nodes are transposed to [D_IN, N] and
owner_masks to [N, B], both regrouped per 3584-node DMA group so every SBUF
partition reads one contiguous run, and cast to the compute dtype. This puts
the contraction dim on partitions for every matmul, so the device does no
transposes at all:

  per 128-node subchunk s (contraction dim on partitions throughout):
    dg[n, 0:512] = nodesT[:, s].T @ [WtT | WgT]     (2 accumulating matmuls)
    gates = sigmoid(dg[:, 256:512] + bg)            (ACT, batched per pair)
    prod  = (dg[:, 0:256] + bt) * gates             (DVE, batched per pair)
    res[b, :] += maskT[:, s].T @ prod               (2 accumulating matmuls
                                                     into persistent PSUM)

Modes: "bf16" (default) stores matmul operands in bf16 (fp32 PSUM accum),
"f32r" uses float32r (tf32-like) storage for ~1000x tighter numerics at
~15-20% lower throughput. Select with BASS_AGG_MODE.
"""

import os
import sys
from contextlib import ExitStack

import numpy as np

sys.path.insert(0, "/opt/trn_rl_repo")

import concourse.bass as bass  # noqa: E402
import concourse.tile as tile  # noqa: E402
from concourse import bacc, mybir  # noqa: E402
from concourse.bass_utils import run_bass_kernel_spmd  # noqa: E402

N, D_IN, D_OUT, B = 200000, 256, 256, 256
NCORES = 8
CHUNK = 128          # nodes per subchunk (one matmul block)
GROUP = 3584         # nodes per DMA group
NSH = 25088          # padded nodes per core (= 196 * 128 = 7 * 3584)
NGROUPS = NSH // GROUP       # 7
SUBS = GROUP // CHUNK        # 28 subchunks per group (even -> 14 pairs)

F32 = mybir.dt.float32
F32R = mybir.dt.float32r
BF16 = mybir.dt.bfloat16

MODE = os.environ.get("BASS_AGG_MODE", "bf16")

_BUILT = {}
_LAST_BG_SCALAR = 1.0


def _build(mode, bg_scalar):
    cdt = BF16 if mode == "bf16" else F32R
    nc = bacc.Bacc("TRN2", target_bir_lowering=False, debug=False,
                   num_devices=NCORES)

    # nodesT grouped: [g][p][k*GROUP + n] = nodesT[k*128+p, g*GROUP+n]
    ndT = nc.dram_tensor("ndT", [NGROUPS, 128, 2 * GROUP], cdt,
                         kind="ExternalInput").ap()
    # maskT grouped: [g][p][s*256 + b] = maskT[g*GROUP + s*128 + p, b]
    mkT = nc.dram_tensor("mkT", [NGROUPS, 128, SUBS * 256], cdt,
                         kind="ExternalInput").ap()
    wf = nc.dram_tensor("wf", [128, 4 * D_OUT], cdt, kind="ExternalInput").ap()
    if bg_scalar is None:
        bfull = nc.dram_tensor("bfull", [128, 1024], F32,
                               kind="ExternalInput").ap()
    else:
        btile = nc.dram_tensor("btile", [128, 256], F32,
                               kind="ExternalInput").ap()
    out_res = nc.dram_tensor("res", [B, D_OUT], F32, kind="ExternalOutput").ap()

    SIG = mybir.ActivationFunctionType.Sigmoid

    with tile.TileContext(nc) as tc, ExitStack() as ctx:
        const = ctx.enter_context(tc.tile_pool(name="const", bufs=1))
        gio = ctx.enter_context(tc.tile_pool(name="gio", bufs=2))
        work = ctx.enter_context(tc.tile_pool(name="work", bufs=3))
        pps = ctx.enter_context(tc.tile_pool(name="pps", bufs=3, space="PSUM"))
        rps = ctx.enter_context(tc.tile_pool(name="rps", bufs=1, space="PSUM"))

        wf_s = const.tile([128, 4 * D_OUT], cdt)
        nc.scalar.dma_start(wf_s[:], wf[:])
        if bg_scalar is None:
            bf_s = const.tile([128, 1024], F32)
            nc.scalar.dma_start(bf_s[:], bfull[:])
        else:
            bt_s = const.tile([128, 256], F32)
            nc.scalar.dma_start(bt_s[:], btile[:])

        res0 = rps.tile([128, D_OUT], F32)
        res1 = rps.tile([128, D_OUT], F32)

        for g in range(NGROUPS):
            # split each group load into n-slices: finer completion
            # granularity -> compute starts sooner, fewer mid-loop stalls.
            # group 0 is split finer so the pipeline fills fast.
            nsp = 8 if g == 0 else 2
            nd_s = gio.tile([128, 2 * GROUP], cdt, tag="nd")
            mk_s = gio.tile([128, SUBS * 256], cdt, tag="mk")
            nd3d = nd_s[:].rearrange("p (k n) -> p k n", k=2)
            ndg = ndT[g].rearrange("p (k n) -> p k n", k=2)
            W = SUBS * 256
            for q in range(nsp):
                lo, hi = q * GROUP // nsp, (q + 1) * GROUP // nsp
                nc.sync.dma_start(nd3d[:, :, lo:hi], ndg[:, :, lo:hi])
                lo, hi = q * W // nsp, (q + 1) * W // nsp
                nc.sync.dma_start(mk_s[:, lo:hi], mkT[g][:, lo:hi])

            for p in range(SUBS // 2):
                s0 = 2 * p
                first = (g == 0 and p == 0)
                last = (g == NGROUPS - 1 and p == SUBS // 2 - 1)

                adt = BF16 if mode == "bf16" else F32
                gt_s = work.tile([128, 512], adt, tag="gts")
                db_s = work.tile([128, 512], adt, tag="dbs")
                if bg_scalar is None:
                    dgb = work.tile([128, 1024], adt, tag="dgb")
                else:
                    dgb = None
                # per-subchunk single-bank psum tiles -> deeper PE pipelining
                for k in range(2):
                    s = s0 + k
                    dg_ps = pps.tile([128, 512], F32, tag="dgp")
                    nc.tensor.matmul(dg_ps[:],
                                     nd_s[:, s * 128:(s + 1) * 128],
                                     wf_s[:, 0:512], start=True, stop=False)
                    nc.tensor.matmul(dg_ps[:],
                                     nd_s[:, GROUP + s * 128:
                                           GROUP + (s + 1) * 128],
                                     wf_s[:, 512:1024], start=False, stop=True)
                    o = k * 256
                    if bg_scalar is None:
                        nc.vector.tensor_add(dgb[:, 2 * o:2 * o + 512],
                                             dg_ps[:], bf_s[:, 0:512])
                        nc.scalar.activation(gt_s[:, o:o + 256],
                                             dgb[:, 2 * o + 256:2 * o + 512],
                                             SIG)
                    else:
                        nc.scalar.activation(gt_s[:, o:o + 256],
                                             dg_ps[:, 256:512], SIG,
                                             bias=float(bg_scalar), scale=1.0)
                        nc.vector.tensor_add(db_s[:, o:o + 256],
                                             dg_ps[:, 0:256], bt_s[:, 0:256])

                pr_s = work.tile([128, 512], cdt, tag="prs")
                if bg_scalar is None:
                    dgb4 = dgb[:].rearrange("q (s h d) -> q s h d", s=2, d=256)
                    pr3 = pr_s[:].rearrange("q (s d) -> q s d", s=2)
                    gt3 = gt_s[:].rearrange("q (s d) -> q s d", s=2)
                    nc.vector.tensor_mul(pr3, dgb4[:, :, 0, :], gt3)
                else:
                    nc.vector.tensor_mul(pr_s[:], db_s[:], gt_s[:])

                for k in range(2):
                    s = s0 + k
                    kfirst = first and k == 0
                    klast = last and k == 1
                    nc.tensor.matmul(res0[:], mk_s[:, s * 256:s * 256 + 128],
                                     pr_s[:, k * 256:(k + 1) * 256],
                                     start=kfirst, stop=klast)
                    nc.tensor.matmul(res1[:],
                                     mk_s[:, s * 256 + 128:s * 256 + 256],
                                     pr_s[:, k * 256:(k + 1) * 256],
                                     start=kfirst, stop=klast)

        rs = work.tile([128, 2 * D_OUT], F32, tag="rout")
        nc.vector.tensor_copy(rs[:, 0:256], res0[:])
        nc.vector.tensor_copy(rs[:, 256:512], res1[:])
        nc.sync.dma_start(out_res[0:128, :], rs[:, 0:256])
        nc.sync.dma_start(out_res[128:256, :], rs[:, 256:512])

    nc.compile()
    return nc


def _get_nc(bg_scalar, mode=None):
    mode = mode or MODE
    key = (mode, None if bg_scalar is None else float(bg_scalar))
    if key not in _BUILT:
        _BUILT[key] = _build(mode, bg_scalar)
    return _BUILT[key]


def _prep_host(nodes, owner_masks, np_cdt):
    """Pad, shard, transpose + regroup nodes/masks into the DMA layouts.
    Cast to the compute dtype first so the big strided copies move half
    the bytes."""
    ntot = NCORES * NSH
    nd = np.zeros((ntot, D_IN), np_cdt)
    nd[:N] = nodes                       # cast f32 -> cdt
    # ndT[c, g, p, k, n] = nodes[c*NSH + g*GROUP + n, k*128 + p]
    ndr = nd.reshape(NCORES, NGROUPS, GROUP, 2, 128)
    ndT = np.ascontiguousarray(ndr.transpose(0, 1, 4, 3, 2)).reshape(
        NCORES, NGROUPS, 128, 2 * GROUP)

    mk = np.zeros((B, ntot), np_cdt)
    mk[:, :N] = owner_masks              # cast int -> cdt (0/1 exact)
    # mkT[c, g, p, s, b] = mask[b, c*NSH + g*GROUP + s*128 + p]
    mkr = mk.reshape(B, NCORES, NGROUPS, SUBS, 128)
    mkT = np.ascontiguousarray(mkr.transpose(1, 2, 4, 3, 0)).reshape(
        NCORES, NGROUPS, 128, SUBS * B)

    return [(ndT[c], mkT[c]) for c in range(NCORES)]


def kernel(nodes, owner_masks, Wt, bt, Wg, bg, _spmd_extra_kwargs=None):
    import ml_dtypes

    nodes = np.asarray(nodes, dtype=np.float32)
    owner_masks = np.asarray(owner_masks)
    Wt = np.asarray(Wt, dtype=np.float32)
    bt = np.asarray(bt, dtype=np.float32)
    Wg = np.asarray(Wg, dtype=np.float32)
    bg = np.asarray(bg, dtype=np.float32)

    bg_scalar = float(bg[0]) if np.all(bg == bg[0]) else None
    global _LAST_BG_SCALAR
    _LAST_BG_SCALAR = bg_scalar
    nc = _get_nc(bg_scalar)
    np_cdt = ml_dtypes.bfloat16 if MODE == "bf16" else np.float32

    shards = _prep_host(nodes, owner_masks, np_cdt)

    # wf: rows = i-chunk features, cols = [WtT | WgT] for chunk0 then chunk1
    WtT, WgT = Wt.T, Wg.T
    wf_np = np.empty((128, 4 * D_OUT), np.float32)
    wf_np[:, 0:256] = WtT[0:128]
    wf_np[:, 256:512] = WgT[0:128]
    wf_np[:, 512:768] = WtT[128:256]
    wf_np[:, 768:1024] = WgT[128:256]
    common = {"wf": wf_np.astype(np_cdt)}
    if bg_scalar is None:
        bfull = np.empty((128, 1024), np.float32)
        for k in range(2):
            bfull[:, k * 512:k * 512 + 256] = bt
            bfull[:, k * 512 + 256:(k + 1) * 512] = bg
        common["bfull"] = bfull
    else:
        common["btile"] = np.ascontiguousarray(
            np.broadcast_to(bt, (128, 256)).astype(np.float32))

    in_maps = [{"ndT": ndTg, "mkT": mkTg, **common}
               for (ndTg, mkTg) in shards]

    extra = _spmd_extra_kwargs or {}
    res = run_bass_kernel_spmd(nc, in_maps, list(range(NCORES)), **extra)
    out = np.zeros((B, D_OUT), np.float64)
    for c in range(NCORES):
        out += res.results[c]["res"].astype(np.float64)
    kernel.last_results = res
    return out.astype(np.float32)

